# revision 21
# baseline (speedup 1.0000x reference)
"""Masked video loss kernel for TRN2 (8 NeuronCores, SPMD).

Algorithmic structure exploited: the decoder input feat_3d is spatially
constant (broadcast of per-frame features over H=W=64), so the three
SAME-padded 3x3x3 convs produce at most 7x7 distinct values per (b,c,t)
(spatial boundary classes at distance 0,1,2,interior,-3,-2,-1 from each
edge). We evaluate the decoder on a 7x7 spatial grid (exact, not an
approximation) and fold the masked MSE through per-class statistics:

  sum_masked (r - o)^2 = r^2 * cnt - 2 r * s1 + s2
     cnt = sum_masked 1, s1 = sum_masked o, s2 = sum_masked o^2

per class. s2 needs no class split (appears fully summed).

Sharding: core = 2*b + th. Each core runs encoder+decoder for batch b
over full T=16 (identical SPMD program), and computes mask stats only
for its t-half (host zeroes the other half of its mask copy).
Host does the final ~10k-flop assembly of the gathered class tensors.
"""

import sys

sys.path.insert(0, "/opt/trn_rl_repo")

from contextlib import ExitStack  # noqa: E402

import numpy as np  # noqa: E402

import concourse.bacc as bacc  # noqa: E402
import concourse.bass as bass  # noqa: E402
import concourse.mybir as mybir  # noqa: E402
import concourse.tile as tile  # noqa: E402
from concourse import bass_utils  # noqa: E402

B, T, C, H, W = 4, 16, 3, 64, 64
D = 256
X = C * H * W  # 12288
HW = H * W  # 4096
NCORES = 8

F32 = mybir.dt.float32
F32R = mybir.dt.float32r
BF16 = mybir.dt.bfloat16
U8 = mybir.dt.uint8

# spatial boundary classes after 3 stacked 3x3 SAME convs
CLS_BOUNDS = [0, 1, 2, 3, H - 3, H - 2, H - 1, H]  # 7 classes
NCLS = 7
# padded 7x7 grid: 9x9 with 1-pixel zero ring; t padded 16 -> 18
GI, GJ = NCLS, NCLS
PI, PJ = GI + 2, GJ + 2  # 9, 9
PT = T + 2  # 18
GJ8 = GJ  # no junk col needed for bf16 matmuls
NVOX_H = 8 * GI * GJ8  # 392 voxels per t-half (8 frames x 7 x 7)


def _emit(nc, a_in, a_out):
    """Emit the per-core program. a_in/a_out: dicts name -> bass.AP."""
    ctx = ExitStack()
    tc = tile.TileContext(nc)
    with tc, ctx:
        io = ctx.enter_context(tc.tile_pool(name="io", bufs=1))
        wenc_pool = ctx.enter_context(tc.tile_pool(name="wenc", bufs=3))
        work = ctx.enter_context(tc.tile_pool(name="work", bufs=1))
        ps_enc = ctx.enter_context(tc.tile_pool(name="ps_enc", bufs=1, space="PSUM"))
        ps = ctx.enter_context(tc.tile_pool(name="ps", bufs=4, space="PSUM"))

        # ---------------- input loads (all host-prepermuted, contiguous) ----
        # big1 columns: [0:1536) obsT | [1536:2048) keepT (1-mask, f32)
        big1 = io.tile([128, 2048], F32)
        nc.sync.dma_start(big1[:], a_in["big1"])
        obsT_sb = big1[:, 0:1536]
        keep = big1[:, 1536:2048]
        # big2 columns: [0:1536) obs_st | [1536:2048) mask_st (f32)
        big2 = io.tile([128, 2048], F32)
        nc.sync.dma_start(big2[:], a_in["big2"])
        O_sb = big2[:, 0:1536]
        mf_st = big2[:, 1536:2048]
        # conv weights: one bf16 tensor [128, 6912+1728+81]
        wc = io.tile([128, 27 * 2 * 128 + 27 * 64 + 27 * 3], BF16)
        nc.sync.dma_start(wc[:], a_in["wconv"])
        w1T_sb = wc[:, 0 : 27 * 2 * 128]
        w2T_sb = wc[:, 27 * 2 * 128 : 27 * 2 * 128 + 27 * 64]
        w3T_sb = wc[0:64, 27 * 2 * 128 + 27 * 64 : 27 * 2 * 128 + 27 * 64 + 27 * 3]
        # consts [128, 35]: [0:2) benc | [2] b1 | [3] b2 (rows<64) | [4] b3 (rows<3)
        #                   [5:19) rhT | [19:35) eye16 (rows<16)
        cons = io.tile([128, 35], F32)
        nc.sync.dma_start(cons[:], a_in["consts"])
        benc_sb = cons[:, 0:2]
        b1_sb = cons[:, 2:3]
        b2_sb = cons[0:64, 3:4]
        b3_sb = cons[0:3, 4:5]
        rhT_sb = cons[:, 5:19]
        eye_sb = cons[0:16, 19:35]

        outv = work.tile([128, 225], F32, tag="outv")
        nc.gpsimd.memset(outv[:], 0.0)

        # ---------------- PE warm-up (HAM clock-gate) during input DMAs ----
        ps_warm = ctx.enter_context(tc.tile_pool(name="ps_warm", bufs=1, space="PSUM"))
        warm_ps = ps_warm.tile([2, 35], F32)
        for i in range(30):
            nc.tensor.matmul(
                warm_ps[:], cons[:, 0:2], cons[:, 0:35], start=(i == 0), stop=(i == 29)
            )
        nc.vector.tensor_copy(outv[96:98, 0:35], warm_ps[:])

        # ---------------- encoder input masking (DVE) ----------------
        xt = work.tile([128, 96 * T], BF16, tag="xt")
        for c in range(C):
            sl = slice(c * 32 * T, (c + 1) * 32 * T)
            nc.vector.tensor_mul(xt[:, sl], obsT_sb[:, sl], keep)

        # ---------------- encoder matmuls: featsT [16, 256] ----------------
        featsT_ps = ps_enc.tile([16, D], F32)
        # stream W_encT in 4 chunks of [128, 24*256]
        for g in range(4):
            wk = wenc_pool.tile([128, 24 * D], BF16, tag="wk")
            nc.sync.dma_start(wk[:], a_in["wencT"][g])
            for r in range(24):
                ki = g * 24 + r
                nc.tensor.matmul(
                    featsT_ps[:],
                    xt[:, ki * T : (ki + 1) * T],
                    wk[:, r * D : (r + 1) * D],
                    start=(ki == 0),
                    stop=(ki == 95),
                )
        featsT_sb = work.tile([16, D], F32, tag="ftsb")
        nc.vector.tensor_copy(featsT_sb[:], featsT_ps[:])
        # transpose to feats [128=(d%128), kd:2, t:16] and add b_enc
        feats_sb = work.tile([128, 2 * T], F32, tag="feats")
        for kd in range(2):
            tr_ps = ps.tile([128, 16], F32, tag="cv")
            nc.tensor.transpose(
                tr_ps[:], featsT_sb[:, kd * 128 : (kd + 1) * 128], eye_sb
            )
            nc.scalar.activation(
                feats_sb[:, kd * T : (kd + 1) * T],
                tr_ps[:],
                mybir.ActivationFunctionType.Identity,
                bias=benc_sb[:, kd : kd + 1],
            )

        # ---------------- broadcast into padded conv input ----------------
        # xpad1 [128, kd:2, t:18, i:9, j:9]
        xpad1 = work.tile([128, 2 * PT * PI * PJ], BF16, tag="xpad1")
        nc.gpsimd.memset(xpad1[:], 0.0)
        v1x = xpad1[:].rearrange("p (kd t i j) -> p kd t i j", kd=2, t=PT, i=PI, j=PJ)
        for kd in range(2):
            src = (
                feats_sb[:, kd * T : (kd + 1) * T]
                .unsqueeze(2)
                .unsqueeze(3)
                .broadcast_to([128, T, GI, GJ])
            )
            nc.vector.tensor_copy(v1x[:, kd, 1 : T + 1, 1 : 1 + GI, 1 : 1 + GJ], src)

        # ---------------- conv1 (+relu) ----------------
        h1_sb = work.tile([128, T * GI * GJ8], F32, tag="h1")
        for hf in range(2):
            p1 = ps.tile([128, NVOX_H], F32, tag="cv")
            first = True
            for kt in range(3):
                for kh in range(3):
                    for kw in range(3):
                        tap = (kt * 3 + kh) * 3 + kw
                        for kd in range(2):
                            rhs = v1x[
                                :,
                                kd,
                                8 * hf + kt : 8 * hf + kt + 8,
                                kh : kh + GI,
                                kw : kw + GJ8,
                            ]
                            nc.tensor.matmul(
                                p1[:],
                                w1T_sb[
                                    :, (tap * 2 + kd) * 128 : (tap * 2 + kd + 1) * 128
                                ],
                                rhs,
                                start=first,
                                stop=(tap == 26 and kd == 1),
                            )
                            first = False
            nc.scalar.activation(
                h1_sb[:, hf * NVOX_H : (hf + 1) * NVOX_H],
                p1[:],
                mybir.ActivationFunctionType.Relu,
                bias=b1_sb[:, 0:1],
            )

        xpad2 = work.tile([128, PT * PI * PJ], BF16, tag="xpad2")
        nc.gpsimd.memset(xpad2[:], 0.0)
        v2x = xpad2[:].rearrange("p (t i j) -> p t i j", t=PT, i=PI, j=PJ)
        nc.vector.tensor_copy(
            v2x[:, 1 : T + 1, 1 : 1 + GI, 1 : 1 + GJ],
            h1_sb[:].rearrange("p (t i j) -> p t i j", t=T, i=GI, j=GJ8)[:, :, :, :GJ],
        )

        # ---------------- conv2 (+relu) ----------------
        h2_sb = work.tile([64, T * GI * GJ8], F32, tag="h2")
        for hf in range(2):
            p2 = ps.tile([64, NVOX_H], F32, tag="cv")
            for kt in range(3):
                for kh in range(3):
                    for kw in range(3):
                        tap = (kt * 3 + kh) * 3 + kw
                        rhs = v2x[
                            :, 8 * hf + kt : 8 * hf + kt + 8, kh : kh + GI, kw : kw + GJ8
                        ]
                        nc.tensor.matmul(
                            p2[:],
                            w2T_sb[:, tap * 64 : (tap + 1) * 64],
                            rhs,
                            start=(tap == 0),
                            stop=(tap == 26),
                        )
            nc.scalar.activation(
                h2_sb[:, hf * NVOX_H : (hf + 1) * NVOX_H],
                p2[:],
                mybir.ActivationFunctionType.Relu,
                bias=b2_sb[:, 0:1],
            )

        xpad3 = work.tile([64, PT * PI * PJ], BF16, tag="xpad3")
        nc.gpsimd.memset(xpad3[:], 0.0)
        v3x = xpad3[:].rearrange("p (t i j) -> p t i j", t=PT, i=PI, j=PJ)
        nc.vector.tensor_copy(
            v3x[:, 1 : T + 1, 1 : 1 + GI, 1 : 1 + GJ],
            h2_sb[:].rearrange("p (t i j) -> p t i j", t=T, i=GI, j=GJ8)[:, :, :, :GJ],
        )

        # ---------------- conv3 (+bias) -> recon classes ----------------
        recon_sb = work.tile([3, T * GI * GJ8], F32, tag="recon")
        for hf in range(2):
            p3 = ps.tile([3, NVOX_H], F32, tag="cv")
            for kt in range(3):
                for kh in range(3):
                    for kw in range(3):
                        tap = (kt * 3 + kh) * 3 + kw
                        rhs = v3x[
                            :, 8 * hf + kt : 8 * hf + kt + 8, kh : kh + GI, kw : kw + GJ8
                        ]
                        nc.tensor.matmul(
                            p3[:],
                            w3T_sb[:, tap * 3 : (tap + 1) * 3],
                            rhs,
                            start=(tap == 0),
                            stop=(tap == 26),
                        )
            nc.scalar.activation(
                recon_sb[:, hf * NVOX_H : (hf + 1) * NVOX_H],
                p3[:],
                mybir.ActivationFunctionType.Identity,
                bias=b3_sb[:, 0:1],
            )
        nc.sync.dma_start(a_out["recon_cls"], recon_sb[:])

        # ---------------- mask stats ----------------
        vO = O_sb.rearrange("p (tt c w) -> p tt c w", tt=8, c=C)
        vM = mf_st.rearrange("p (tt w) -> p tt w", tt=8)
        mo = work.tile([128, 8 * C * W], F32, tag="mo")
        vmo = mo[:].rearrange("p (tt c w) -> p tt c w", tt=8, c=C)
        for c in range(C):
            nc.vector.tensor_mul(vmo[:, :, c, :], vO[:, :, c, :], vM[:])
        mo2 = work.tile([128, 8 * C * W], F32, tag="mo2")
        nc.vector.tensor_mul(mo2[:], mo[:], O_sb)
        # s2 partial (sum over everything later on host)
        nc.vector.reduce_sum(outv[:, 224:225], mo2[:], axis=mybir.AxisListType.X)
        # w-class segmented reduce: U1 [128, (tt,c,j)], Uc [128, (tt,j)]
        U1 = work.tile([128, 8 * C * NCLS], F32, tag="U1")
        vU1 = U1[:].rearrange("p (tt c j) -> p tt c j", tt=8, c=C)
        Uc = work.tile([128, 8 * NCLS], F32, tag="Uc")
        vUc = Uc[:].rearrange("p (tt j) -> p tt j", tt=8)
        vmo4 = mo[:].rearrange("p (tt c w) -> p tt c w", tt=8, c=C)
        for j in range(NCLS):
            w0, w1_ = CLS_BOUNDS[j], CLS_BOUNDS[j + 1]
            nc.vector.reduce_sum(
                vU1[:, :, :, j], vmo4[:, :, :, w0:w1_], axis=mybir.AxisListType.X
            )
            nc.vector.reduce_sum(
                vUc[:, :, j], vM[:, :, w0:w1_], axis=mybir.AxisListType.X
            )
        # h-class reduce via PE: V1 [14, 168], Vc [14, 56]
        pv1 = ps.tile([14, 8 * C * NCLS], F32, tag="cv")
        nc.tensor.matmul(pv1[:], rhT_sb, U1[:], start=True, stop=True)
        nc.vector.tensor_copy(outv[0:14, 0:168], pv1[:])
        pvc = ps.tile([14, 8 * NCLS], F32, tag="cv")
        nc.tensor.matmul(pvc[:], rhT_sb, Uc[:], start=True, stop=True)
        nc.vector.tensor_copy(outv[0:14, 168:224], pvc[:])
        nc.sync.dma_start(a_out["outv"], outv[:])


_CACHE = {}


def _build():
    if "nc" in _CACHE:
        return _CACHE["nc"]
    nc = bacc.Bacc("TRN2", target_bir_lowering=False, debug=False)
    a_in = {}

    def din(name, shape, dt):
        a_in[name] = nc.dram_tensor(name, shape, dt, kind="ExternalInput").ap()

    din("big1", (128, 2048), F32)
    din("big2", (128, 2048), F32)
    din("wencT", (4, 128, 24 * D), BF16)
    din("wconv", (128, 27 * 2 * 128 + 27 * 64 + 27 * 3), BF16)
    din("consts", (128, 35), F32)
    a_out = {}
    for name, shape in [
        ("recon_cls", (3, T * GI * GJ8)),
        ("outv", (128, 225)),
    ]:
        a_out[name] = nc.dram_tensor(name, shape, F32, kind="ExternalOutput").ap()
    _emit(nc, a_in, a_out)
    nc.compile()
    _CACHE["nc"] = nc
    return nc


def make_in_maps(obs_strip, mask, W_enc, b_enc, w1, b1, w2, b2, w3, b3):
    import ml_dtypes

    bf16 = ml_dtypes.bfloat16
    obs_strip = np.ascontiguousarray(obs_strip, dtype=np.float32)
    mask_f = np.ascontiguousarray(mask).astype(np.float32)
    rh = np.zeros((7, 64), np.float32)
    for i in range(NCLS):
        rh[i, CLS_BOUNDS[i] : CLS_BOUNDS[i + 1]] = 1.0
    rhT = np.zeros((128, 14), np.float32)
    for u in range(2):
        rhT[u * 64 : (u + 1) * 64, u * 7 : (u + 1) * 7] = rh.T
    consts = np.zeros((128, 35), np.float32)
    consts[:, 0:2] = np.asarray(b_enc, np.float32).reshape(2, 128).T
    consts[:, 2] = np.asarray(b1, np.float32)
    consts[0:64, 3] = np.asarray(b2, np.float32)
    consts[0:3, 4] = np.asarray(b3, np.float32)
    consts[:, 5:19] = rhT
    consts[0:16, 19:35] = np.eye(16, dtype=np.float32)
    wconv = np.zeros((128, 27 * 2 * 128 + 27 * 64 + 27 * 3), bf16)
    wconv[:, 0 : 27 * 2 * 128] = (
        np.ascontiguousarray(w1)
        .transpose(2, 3, 4, 1, 0)
        .reshape(27, 2, 128, 128)
        .transpose(2, 0, 1, 3)
        .reshape(128, 27 * 2 * 128)
        .astype(bf16)
    )
    wconv[:, 27 * 2 * 128 : 27 * 2 * 128 + 27 * 64] = (
        np.ascontiguousarray(w2)
        .transpose(2, 3, 4, 1, 0)
        .reshape(27, 128, 64)
        .transpose(1, 0, 2)
        .reshape(128, 27 * 64)
        .astype(bf16)
    )
    wconv[0:64, 27 * 2 * 128 + 27 * 64 :] = (
        np.ascontiguousarray(w3)
        .transpose(2, 3, 4, 1, 0)
        .reshape(27, 64, 3)
        .transpose(1, 0, 2)
        .reshape(64, 27 * 3)
        .astype(bf16)
    )
    wencT = np.ascontiguousarray(
        np.asarray(W_enc, np.float32)
        .T.reshape(4, 24, 128, D)
        .transpose(0, 2, 1, 3)
        .reshape(4, 128, 24 * D)
    ).astype(bf16)
    shared = {"wencT": wencT, "wconv": wconv, "consts": consts}
    in_maps = []
    for core in range(NCORES):
        b, th = core // 2, core % 2
        mask_st = mask_f[b].copy()
        if th == 0:
            mask_st[8:] = 0.0
        else:
            mask_st[:8] = 0.0
        big1 = np.empty((128, 2048), np.float32)
        big1[:, 0:1536] = (
            obs_strip[b].reshape(T, 96, 128).transpose(2, 1, 0).reshape(128, 96 * T)
        )
        big1[:, 1536:2048] = 1.0 - mask_f[b].reshape(T, 32, 128).transpose(
            2, 1, 0
        ).reshape(128, 32 * T)
        big2 = np.empty((128, 2048), np.float32)
        big2[:, 0:1536] = (
            obs_strip[b]
            .reshape(8, 2, C, H, W)
            .transpose(1, 3, 0, 2, 4)
            .reshape(128, 8 * C * W)
        )
        big2[:, 1536:2048] = mask_st.reshape(8, 2, H, W).transpose(1, 2, 0, 3).reshape(
            128, 8 * W
        )
        in_maps.append({"big1": big1, "big2": big2, **shared})
    return in_maps


def assemble(results):
    total_sq = 0.0
    total_cnt = 0.0
    for core in range(NCORES):
        r = results[core]
        recon = r["recon_cls"].astype(np.float64).reshape(3, T, GI, GJ8)[..., :GJ]
        outv = r["outv"].astype(np.float64)
        v1 = outv[0:14, 0:168].reshape(2, NCLS, 8, C, NCLS)  # [u,i,tt,c,j]
        vc = outv[0:14, 168:224].reshape(2, NCLS, 8, NCLS)  # [u,i,tt,j]
        s2 = float(outv[:, 224].sum())
        s1 = np.zeros((T, C, NCLS, NCLS))
        cnt = np.zeros((T, NCLS, NCLS))
        for u in range(2):
            s1[u::2] = v1[u].transpose(1, 2, 0, 3)  # [tt,c,i,j]
            cnt[u::2] = vc[u].transpose(1, 0, 2)  # [tt,i,j]
        rt = recon.transpose(1, 0, 2, 3)  # [t,c,i,j]
        total_sq += float((rt * rt * cnt[:, None]).sum() - 2.0 * (rt * s1).sum() + s2)
        total_cnt += float(cnt.sum())
    loss = total_sq / max(total_cnt * C, 1.0)
    return np.float32(loss)


def kernel(**inputs):
    nc = _build()
    in_maps = make_in_maps(**inputs)
    res = bass_utils.run_bass_kernel_spmd(nc, in_maps, core_ids=list(range(NCORES)))
    _CACHE["last_res"] = res
    return assemble(res.results)


if __name__ == "__main__":
    pass
